# revision 17
# baseline (speedup 1.0000x reference)
"""MaxRecallLoss Trainium2 kernel: 8-core class-sharded Bass/Tile implementation.

Full inputs in, full (scalar) output out. Sharding strategy: rows are
class-sorted on the host and each NeuronCore processes one target class
(counts are ~B/8 per class, so this is balanced data parallelism with a
class-aligned partition). Rows are padded to a fixed per-core size with
zero-logit rows; padding contributions are measured on-device (partition 127
is entirely padding) and subtracted exactly on the host. Per-class constants
(label-smoothing coefficients, FN/hard-mining multipliers, cancer membership)
enter as [P,1] per-partition scalars, so one SPMD program serves all cores.

Device math (per core, all classes in a row co-resident in one partition):
  u = exp(x/1.5) (bf16, class-major) -> sum/product trees give E15, Ec, Enc,
  P8=prod(u) and Pc (so S = 1.5*ln P8, Sc = 1.5*ln Pc), u_t is a slot slice,
  one batched Ln activation produces all per-row logs, argmax flags from
  bf16 max trees, CE/multiplier algebra in a few fused vector ops, and the
  soft-recall probability via p = 1/(1+exp(-1.5*(ln Ec - ln Enc))) with a
  fast reciprocal. Weighted sums accumulate via accum_out; the host applies
  base_weight (runtime class_counts) and the final recall ratio.
"""
import os
import sys

try:
    import concourse.bass as bass  # noqa: F401
except ImportError:
    sys.path.insert(0, "/opt/trn_rl_repo")

import numpy as np

import concourse.bass as bass
import concourse.tile as tile
from concourse import bacc, mybir
from concourse.bass_utils import run_bass_kernel_spmd

F32 = mybir.dt.float32
BF16 = mybir.dt.bfloat16
ALU = mybir.AluOpType
ACTF = mybir.ActivationFunctionType

B = 2097152
C = 8
NCORES = 8
P = 128
RPP = 2112                 # padded rows per partition per core
PADN = P * RPP             # 270336 padded rows per core
NCHUNK = 2
RT = RPP // NCHUNK         # 1056 rows per partition per chunk

CANCER = (0, 1, 3)
TEMP = 1.5
CSM, BSM = 0.05, 0.1
NORM_B = 1.05
RECALL_W = 0.5

REPEAT = int(os.environ.get("KREPEAT", "1"))

_NC = None


def _class_consts(c):
    """Per-core constants, laid out as cst[:, i] columns."""
    isc = 1.0 if c in CANCER else 0.0
    if isc:
        cs = (CSM / C) / TEMP
        ct = (1.0 - CSM) / TEMP
        cc = 0.0
    else:
        cs = (BSM / C) / NORM_B / TEMP
        ct = (1.0 - BSM) / NORM_B / TEMP
        cc = (BSM * 0.5 / 3.0) / NORM_B / TEMP
    KK = 5.0 + (4.0 if c == 0 else 0.0)
    return np.array([
        1.0 if isc else 0.0,       # 0: cA  (u_t slot0 coefficient)
        0.0 if isc else 1.0,       # 1: cB  (u_t slot2 coefficient)
        -1.5 * cs,                 # 2: ncs (applied to ln P8)
        -1.5 * cc,                 # 3: ncc (applied to ln Pc)
        -1.5 * ct,                 # 4: nct (applied to ln u_t)
        1.0 - KK,                  # 5: BK
        1.0 + isc * KK,            # 6: c6p (g = isc*t1 + c6p)
        isc,                       # 7: isc
    ], dtype=np.float32)


def _body(nc, tc, xin, cstin, out):
    import contextlib
    ctx = contextlib.ExitStack()
    with ctx:
        singles = ctx.enter_context(tc.tile_pool(name="singles", bufs=1))
        xpool = ctx.enter_context(tc.tile_pool(name="xpool", bufs=2))
        upool = ctx.enter_context(tc.tile_pool(name="upool", bufs=2))
        lpool = ctx.enter_context(tc.tile_pool(name="lpool", bufs=1))
        tmp = ctx.enter_context(tc.tile_pool(name="tmp", bufs=1))

        xg = xin.rearrange("c (p r) -> p c r", p=P)      # [128, 8, RPP]

        cst = singles.tile([P, 8], F32)
        nc.sync.dma_start(cst[:], cstin[:, :])
        stats = singles.tile([P, 8], F32)
        nc.vector.memset(stats[:], 0.0)

        def SC(i):
            return cst[:, i:i + 1]

        def _chunks():
            for k in range(NCHUNK):
                X = xpool.tile([P, C, RT], F32, tag="x", name="x")
                nc.sync.dma_start(X[:], xg[:, :, k * RT:(k + 1) * RT])

                # class-major exp: u[c, r] = exp(x[c, r] / 1.5), bf16
                u = upool.tile([P, C, RT], BF16, tag="u", name="u")
                nc.scalar.activation(u[:], X[:], ACTF.Exp, scale=1.0 / TEMP)

                def uc(c):
                    return u[:, c, :]

                lnin = lpool.tile([P, 4, RT], BF16, tag="lnin", name="lnin")
                lnout = lpool.tile([P, 4, RT], F32, tag="lnout", name="lnout")

                def T(name, ch=1, dt=BF16):
                    if ch == 1:
                        return tmp.tile([P, RT], dt, tag=name, name=name)
                    return tmp.tile([P, ch, RT], dt, tag=name, name=name)

                # E-block: sums. Ec/Enc kept for the soft-recall ratio.
                Ec = T("Ec")
                Enc = T("Enc")
                e1 = T("e1")
                nc.vector.tensor_add(e1[:], uc(0), uc(1))
                nc.vector.tensor_add(Ec[:], e1[:], uc(3))
                s2 = T("s2", 2)
                nc.vector.tensor_add(s2[:], u[:, 4:6, :], u[:, 6:8, :])
                s3 = T("s3")
                nc.vector.tensor_add(s3[:], s2[:, 0, :], s2[:, 1, :])
                nc.vector.tensor_add(Enc[:], s3[:], uc(2))
                nc.vector.tensor_add(lnin[:, 0, :], Ec[:], Enc[:])     # E15

                # P-block: products -> lnin slots 1 (P8), 2 (Pc)
                p1 = T("p1")
                nc.vector.tensor_mul(p1[:], uc(0), uc(1))
                nc.vector.tensor_mul(lnin[:, 2, :], p1[:], uc(3))      # Pc
                q2 = T("q2", 2)
                nc.vector.tensor_mul(q2[:], u[:, 4:6, :], u[:, 6:8, :])
                q3 = T("q3")
                nc.vector.tensor_mul(q3[:], q2[:, 0, :], q2[:, 1, :])
                pn = T("pn")
                nc.vector.tensor_mul(pn[:], q3[:], uc(2))              # Pnc
                nc.vector.tensor_mul(lnin[:, 1, :], lnin[:, 2, :], pn[:])  # P8

                # u_t = cA*u0 + cB*u2 -> lnin slot 3
                ta = T("ta")
                nc.vector.tensor_scalar(ta[:], uc(0), SC(0), None, op0=ALU.mult)
                nc.vector.scalar_tensor_tensor(
                    lnin[:, 3, :], in0=uc(2), scalar=SC(1), in1=ta[:],
                    op0=ALU.mult, op1=ALU.add)

                # batched ln over all 4 slots
                nc.scalar.activation(lnout.rearrange("p s r -> p (s r)"),
                                     lnin.rearrange("p s r -> p (s r)"),
                                     ACTF.Ln)

                # maxes and argmax flags (bf16, monotone under exp)
                m2 = T("m2", 2)
                nc.vector.tensor_tensor(m2[:], u[:, 4:6, :], u[:, 6:8, :],
                                        op=ALU.max)
                m3 = T("m3")
                nc.vector.tensor_tensor(m3[:], m2[:, 0, :], m2[:, 1, :],
                                        op=ALU.max)
                Mnc = T("Mnc")
                nc.vector.tensor_tensor(Mnc[:], m3[:], uc(2), op=ALU.max)
                mc1 = T("mc1")
                nc.vector.tensor_tensor(mc1[:], uc(0), uc(1), op=ALU.max)
                Mc = T("Mc")
                nc.vector.tensor_tensor(Mc[:], mc1[:], uc(3), op=ALU.max)
                icp = T("icp")
                nc.vector.tensor_tensor(icp[:], Mc[:], Mnc[:], op=ALU.is_ge)
                e2 = T("e2")
                nc.vector.tensor_tensor(e2[:], lnin[:, 3, :], Mc[:],
                                        op=ALU.is_ge)

                # g = isc*(icp*((1-KK)-e2)) + c6p  (c6p = 1+isc*KK)
                Bt = T("Bt")
                nc.vector.tensor_scalar(Bt[:], e2[:], -1.0, SC(5),
                                        op0=ALU.mult, op1=ALU.add)
                t1 = T("t1")
                nc.vector.tensor_mul(t1[:], icp[:], Bt[:])
                g = T("g")
                nc.vector.tensor_scalar(g[:], t1[:], SC(7), SC(6),
                                        op0=ALU.mult, op1=ALU.add)

                # ce = lse + ncs*lnP8 + ncc*lnPc + nct*ln u_t  (f32 chain)
                ce1 = T("ce1", dt=F32)
                nc.vector.scalar_tensor_tensor(
                    ce1[:], in0=lnout[:, 1, :], scalar=SC(2),
                    in1=lnout[:, 0, :], op0=ALU.mult, op1=ALU.add)
                ce2 = T("ce2", dt=F32)
                nc.vector.scalar_tensor_tensor(
                    ce2[:], in0=lnout[:, 2, :], scalar=SC(3), in1=ce1[:],
                    op0=ALU.mult, op1=ALU.add)
                ce3 = T("ce3")
                nc.vector.scalar_tensor_tensor(
                    ce3[:], in0=lnout[:, 3, :], scalar=SC(4), in1=ce2[:],
                    op0=ALU.mult, op1=ALU.add)

                # gP = g * ce, accumulated into stats[:, k]
                gP = T("icp")  # icp's buffer is dead by now
                nc.vector.scalar_tensor_tensor(
                    gP[:], in0=g[:], scalar=1.0, in1=ce3[:],
                    op0=ALU.mult, op1=ALU.mult,
                    accum_out=stats[:, k:k + 1])

                # soft recall: p = Ec / E15 (temp-1.5 cancer probability)
                Ef = T("Ef", dt=F32)
                nc.vector.tensor_copy(Ef[:], lnin[:, 0, :])
                r = T("r", dt=F32)
                nc.vector.reciprocal_approx_fast(r[:], Ef[:])
                pj = T("Ef", dt=F32)  # Ef dead after reciprocal
                nc.vector.scalar_tensor_tensor(
                    pj[:], in0=Ec[:], scalar=SC(7), in1=r[:],
                    op0=ALU.mult, op1=ALU.mult,
                    accum_out=stats[:, 2 + k:3 + k])

        if REPEAT > 1:
            with tc.For_i(0, REPEAT, 1) as _rep:
                _chunks()
        else:
            _chunks()

        nc.sync.dma_start(out[:, :], stats[:])


def _build():
    nc = bacc.Bacc("TRN2", target_bir_lowering=False, debug=False,
                   num_devices=NCORES)
    xin = nc.dram_tensor("x", [C, PADN], F32, kind="ExternalInput").ap()
    cstin = nc.dram_tensor("cst", [P, 8], F32, kind="ExternalInput").ap()
    out = nc.dram_tensor("out", [P, 8], F32, kind="ExternalOutput").ap()
    with tile.TileContext(nc) as tc:
        _body(nc, tc, xin, cstin, out)
    nc.compile()
    return nc


def get_nc():
    global _NC
    if _NC is None:
        _NC = _build()
    return _NC


def make_in_maps(logits, targets):
    """Host-side shard: class-sort rows, one class per core, column-swapped
    so the target class sits at slot 0 (cancer cores) or slot 2 (benign)."""
    order = np.argsort(targets, kind="stable")
    counts = np.bincount(targets, minlength=C)
    starts = np.zeros(C + 1, np.int64)
    starts[1:] = np.cumsum(counts)
    # partition 127 (rows >= (P-1)*RPP) must be pure padding
    assert counts.max() <= (P - 1) * RPP, counts

    in_maps = []
    for c in range(C):
        rows = order[starts[c]:starts[c + 1]]
        xc = logits[rows]
        perm = list(range(C))
        if c in CANCER:
            perm[0], perm[c] = perm[c], perm[0]
        else:
            perm[2], perm[c] = perm[c], perm[2]
        xp = np.zeros((C, PADN), np.float32)
        xp[:, :len(rows)] = xc[:, perm].T
        cstv = np.broadcast_to(_class_consts(c), (P, 8)).copy()
        in_maps.append({"x": xp, "cst": cstv})
    return in_maps, counts


def kernel(logits, targets, class_counts):
    logits = np.ascontiguousarray(np.asarray(logits, dtype=np.float32))
    targets = np.ascontiguousarray(np.asarray(targets, dtype=np.int32))
    cc = np.asarray(class_counts, dtype=np.float32)

    w = 1.0 / np.sqrt(cc.astype(np.float64) + 1.0)
    bw = w / w.sum() * C

    nc = get_nc()
    in_maps, counts = make_in_maps(logits, targets)
    res = run_bass_kernel_spmd(nc, in_maps, core_ids=list(range(NCORES)))

    wce = 0.0
    tp = 0.0
    for c in range(NCORES):
        st = res.results[c]["out"].astype(np.float64)
        sum_g = st[:, 0].sum() + st[:, 1].sum()
        pad_g = (st[127, 0] + st[127, 1]) / RPP
        n_pad = PADN - counts[c]
        wce += bw[c] * (sum_g - n_pad * pad_g)
        if c in CANCER:
            sum_p = st[:, 2].sum() + st[:, 3].sum()
            pad_p = (st[127, 2] + st[127, 3]) / RPP
            tp += sum_p - n_pad * pad_p
    cnt = int(counts[0] + counts[1] + counts[3])
    base = wce / B
    recall = tp / cnt
    out = base + RECALL_W * (1.0 - recall)
    return np.float32(out)
